# revision 1
# baseline (speedup 1.0000x reference)
"""GPT forward pass on 8 Trainium2 NeuronCores — sequence-parallel (SP8).

Model: B=2, S=1024, D=1024, H=16 heads (hd=64), L=6 layers, V=50257,
tied embedding head.

Sharding: the 2048 tokens are split into 16 causal blocks of 128;
core c (group g=c//4 over batch, rank r=c%4) owns query blocks
A=r and B=7-r of batch g (256 tokens), which balances causal attention
work (each core covers 9 of the 36 (q,k) block pairs plus 3 masked-out
slack slots to keep the SPMD instruction stream uniform).

Every core holds the FULL weights (bf16) and computes full-D
activations for its own tokens, so LN / QKV / Wo / FFN need NO
collectives.  The only per-layer communication is a bf16 AllGather of
K and V (~1 MB/core) within each 4-core group.  The final LN output is
AllGathered once before the vocab-sharded tied-logit matmul.

On-device layout: activations feature-major (x[p, kt, t] = X^T),
weights pre-sliced on the host into contiguous [128, m, kt, 128]
strips, V produced token-major directly (activation-stationary
matmul), so there are ZERO on-device transposes.  Scores are computed
transposed ([k, q]) so softmax normalization reduces to a ones-column
in the AV matmul; causality is a host-built 0/1 mask multiplied into
exp(scores) (uniform instruction stream across cores).
"""

import sys

sys.path.insert(0, "/opt/trn_rl_repo")

import contextlib

import numpy as np
import ml_dtypes

import concourse.bacc as bacc
import concourse.mybir as mybir
import concourse.tile as tile
from concourse.bass import ts
from concourse.bass_utils import run_bass_kernel_spmd

F32 = mybir.dt.float32
F32R = mybir.dt.float32r
BF16 = mybir.dt.bfloat16
AF = mybir.ActivationFunctionType
ALU = mybir.AluOpType
BF16NP = ml_dtypes.bfloat16

# Model dims
B, S, D, H, L, V = 2, 1024, 1024, 16, 6, 50257
HD = D // H            # 64
DFF = 4 * D            # 4096
N_CORES = 8
G = 4                  # group size (cores per batch element)
KD = D // 128          # 8 feature tiles
HC = H // 2            # 8 head-chunks (2 heads per 128 partitions)
TPC = 256              # tokens per core
BLK = 128              # token block
M1 = DFF // 128        # 32 w1 out strips
M2 = KD                # 8 w2 out strips
NSLOT = 12             # attention slots per head (4 A-side + 8 B-side)
VS = 12565             # vocab rows per group-rank (last: 12562)
VSP = 12800            # padded
NVB = VSP // 128       # 100 vocab blocks
T = 1024               # tokens per group (gathered)

KCOLS = HC * TPC            # 2048 k cols in kv contribution
VSEG = H * (HD + 1)         # 1040 v cols per token block (ones col incl.)
KVCOLS = KCOLS + 2 * VSEG   # 4128
NFCOLS = KD * TPC           # 2048

# key block kb -> (rank, slot-within-rank)
RHO = [kb if kb < 4 else 7 - kb for kb in range(8)]
SIG = [0 if kb < 4 else 1 for kb in range(8)]

REPLICA_GROUPS = [[0, 1, 2, 3], [4, 5, 6, 7]]


def _f(name, l=None):
    return name if l is None else f"{name}{l}"


def build_program(debug_taps=False):
    nc = bacc.Bacc("TRN2", target_bir_lowering=False, debug=False,
                   enable_asserts=True, num_devices=N_CORES)

    inp = {}

    def dram_in(name, shape, dtype=BF16):
        inp[name] = nc.dram_tensor(name, shape, dtype, kind="ExternalInput").ap()
        return inp[name]

    dram_in("x0", [128, KD, TPC], F32R)
    dram_in("ones", [128, 1], F32R)
    dram_in("m01", [128, NSLOT, BLK], BF16)
    for l in range(L):
        dram_in(_f("wq", l), [128, KD, KD, 128])    # [p, m, kt, 128]
        dram_in(_f("wk", l), [128, KD, KD, 128])
        dram_in(_f("wv", l), [128, KD, D])          # plain k-fold (moving)
        dram_in(_f("wo", l), [128, KD, KD, 128])
        dram_in(_f("w1", l), [128, M1, KD, 128])
        dram_in(_f("w2", l), [128, M2, M1, 128])
        dram_in(_f("b1", l), [128, M1], F32)
        dram_in(_f("b2", l), [128, M2], F32)
    dram_in("emb", [128, NVB, KD, 128])
    logits = nc.dram_tensor("logits", [VSP, T], F32, kind="ExternalOutput").ap()

    taps = {}
    if debug_taps:
        for name in ["n1_0", "q_0", "o_0", "xa_0", "x_1", "nf"]:
            taps[name] = nc.dram_tensor("dbg_" + name, [128, KD, TPC], F32,
                                        kind="ExternalOutput").ap()

    with tile.TileContext(nc) as tc:
        _body(tc, inp, logits, taps)
    nc.compile()
    return nc


def _body(tc, inp, logits, taps):
    nc = tc.nc
    ctx = contextlib.ExitStack()
    with ctx:
        # --- SBUF pools ---
        singles = ctx.enter_context(tc.tile_pool(name="singles", bufs=1))
        xp = ctx.enter_context(tc.tile_pool(name="xp", bufs=1))        # 8K
        npool = ctx.enter_context(tc.tile_pool(name="npool", bufs=2))  # 8K
        tmp = ctx.enter_context(tc.tile_pool(name="tmp", bufs=2))      # 2K
        qp = ctx.enter_context(tc.tile_pool(name="qp", bufs=1))        # 4K
        stg = ctx.enter_context(tc.tile_pool(name="stg", bufs=1))      # 8.25K
        kvp = ctx.enter_context(tc.tile_pool(name="kvp", bufs=1))      # 33K
        op = ctx.enter_context(tc.tile_pool(name="op", bufs=1))        # 4K
        hp = ctx.enter_context(tc.tile_pool(name="hp", bufs=1))        # 16K
        wq_p = ctx.enter_context(tc.tile_pool(name="wq_p", bufs=8))    # 12K
        wv_p = ctx.enter_context(tc.tile_pool(name="wv_p", bufs=1))    # 16K
        w2_p = ctx.enter_context(tc.tile_pool(name="w2_p", bufs=4))    # 32K
        embp = ctx.enter_context(tc.tile_pool(name="embp", bufs=4))    # 8K
        ep = ctx.enter_context(tc.tile_pool(name="ep", bufs=5))        # 15K
        stat = ctx.enter_context(tc.tile_pool(name="stat", bufs=2))
        bcp = ctx.enter_context(tc.tile_pool(name="bcp", bufs=2))      # 2K
        lout = ctx.enter_context(tc.tile_pool(name="lout", bufs=3))    # 6K
        bias = ctx.enter_context(tc.tile_pool(name="bias", bufs=2))
        nfp = ctx.enter_context(tc.tile_pool(name="nfp", bufs=1))      # 16K
        # --- PSUM pools (8 banks) ---
        ps_mm = ctx.enter_context(tc.tile_pool(name="ps_mm", bufs=4, space="PSUM"))
        ps_po = ctx.enter_context(tc.tile_pool(name="ps_po", bufs=2, space="PSUM"))
        ps_st = ctx.enter_context(tc.tile_pool(name="ps_st", bufs=1, space="PSUM"))
        # --- DRAM (collective bounce) ---
        dram = ctx.enter_context(tc.tile_pool(name="dram", bufs=2, space="DRAM"))

        # --- constants / persistent ---
        ones_t = singles.tile([128, 1], F32R)
        nc.sync.dma_start(out=ones_t[:], in_=inp["ones"][:])
        eps_t = singles.tile([1, 1], F32)
        nc.vector.memset(eps_t[:], 1e-5)
        m01t = singles.tile([128, NSLOT, BLK], BF16)
        nc.sync.dma_start(out=m01t[:], in_=inp["m01"][:])

        xt = xp.tile([128, KD, TPC], F32R, tag="x")
        nc.sync.dma_start(out=xt[:], in_=inp["x0"][:])

        def ln_alloc(nm):
            s1 = ps_st.tile([1, TPC], F32, tag="st1", name=f"s1{nm}")
            s2 = ps_st.tile([1, TPC], F32, tag="st2", name=f"s2{nm}")
            return s1, s2

        def ln_stats_k(src, s1, s2, k):
            nc.tensor.matmul(s1[:], ones_t[:], src[:, k, :],
                             start=(k == 0), stop=(k == KD - 1))
            sq = tmp.tile([128, TPC], F32R, tag="tmp", name=f"sq_{k}")
            nc.vector.tensor_tensor(
                out=sq[:], in0=src[:, k, :].bitcast(F32),
                in1=src[:, k, :].bitcast(F32), op=ALU.mult)
            nc.tensor.matmul(s2[:], ones_t[:], sq[:],
                             start=(k == 0), stop=(k == KD - 1))

        def ln_finish(src, dst, s1, s2):
            m = stat.tile([1, TPC], F32, tag="sa")
            nc.vector.tensor_scalar_mul(m[:], s1[:], 1.0 / D)
            msq = stat.tile([1, TPC], F32, tag="sb")
            nc.vector.tensor_tensor(out=msq[:], in0=m[:], in1=m[:], op=ALU.mult)
            var = stat.tile([1, TPC], F32, tag="sb")
            nc.vector.scalar_tensor_tensor(
                out=var[:], in0=s2[:], scalar=1.0 / D, in1=msq[:],
                op0=ALU.mult, op1=ALU.subtract)
            rs = stat.tile([1, TPC], F32, tag="sb")
            nc.scalar.activation(rs[:], var[:], AF.Sqrt, bias=eps_t[:])
            nc.vector.reciprocal(rs[:], rs[:])
            a = stat.tile([1, TPC], F32, tag="sa")
            nc.vector.scalar_tensor_tensor(
                out=a[:], in0=m[:], scalar=-1.0, in1=rs[:],
                op0=ALU.mult, op1=ALU.mult)
            rB = bcp.tile([128, TPC], F32, tag="rB")
            nc.gpsimd.partition_broadcast(rB[:], rs[:])
            aB = bcp.tile([128, TPC], F32, tag="aB")
            nc.gpsimd.partition_broadcast(aB[:], a[:])
            for k in range(KD):
                t2 = tmp.tile([128, TPC], F32, tag="tmp")
                nc.vector.tensor_tensor(
                    out=t2[:], in0=src[:, k, :].bitcast(F32), in1=rB[:],
                    op=ALU.mult)
                nc.vector.tensor_tensor(
                    out=dst[:, k, :], in0=t2[:], in1=aB[:], op=ALU.add)

        def layer_norm(src, dst, nm="p"):
            s1, s2 = ln_alloc(nm)
            for k in range(KD):
                ln_stats_k(src, s1, s2, k)
            ln_finish(src, dst, s1, s2)

        def k_stage_gather(l, n1):
            """K (feature-major) -> staging -> AllGather."""
            stage = stg.tile([128, KCOLS], BF16, tag="stgk", name=f"stagek{l}")
            for mp in range(KD // 2):
                pk = ps_mm.tile([128, 2, TPC], F32, tag="mm",
                                name=f"pk{l}_{mp}")
                for j in range(2):
                    m = 2 * mp + j
                    wkt = wq_p.tile([128, KD, 128], BF16, tag="wqk",
                                    name=f"wk{l}_{m}")
                    nc.sync.dma_start(out=wkt[:], in_=inp[_f("wk", l)][:, m])
                    for k in range(KD):
                        nc.tensor.matmul(pk[:, j, :], wkt[:, k, :],
                                         n1[:, k, :],
                                         start=(k == 0), stop=(k == KD - 1))
                nc.vector.tensor_scalar_mul(
                    stage[:, ts(mp, 2 * TPC)], pk[:], 1.0)
            ag_in = dram.tile([128, KCOLS], BF16, tag="agik", name=f"agik{l}")
            ag_out = dram.tile([G, 128, KCOLS], BF16, tag="agok",
                               name=f"agok{l}")
            nc.sync.dma_start(out=ag_in[:], in_=stage[:])
            nc.gpsimd.collective_compute(
                "AllGather", ALU.bypass, replica_groups=REPLICA_GROUPS,
                ins=[ag_in.opt()], outs=[ag_out.opt()])
            return ag_out

        def v_stage_gather(l, n1):
            """V (token-major, + ones cols) -> staging -> AllGather."""
            stage = stg.tile([128, 2 * VSEG], BF16, tag="stgv",
                             name=f"stagev{l}")
            ones_view = stage[:].rearrange(
                "p (b h c) -> p b h c", b=2, h=H)[:, :, :, HD:]
            nc.vector.memset(ones_view, 1.0)
            wvt = wv_p.tile([128, KD, D], BF16, tag="wv", name=f"wv{l}")
            nc.sync.dma_start(out=wvt[:], in_=inp[_f("wv", l)][:])
            for blk in range(2):
                for hh in range(2):   # halves of the head dim (512 cols)
                    pv = ps_mm.tile([128, 512], F32, tag="mm",
                                    name=f"pv{l}_{blk}_{hh}")
                    for k in range(KD):
                        nc.tensor.matmul(
                            pv[:], n1[:, k, ts(blk, BLK)],
                            wvt[:, k, ts(hh, 512)],
                            start=(k == 0), stop=(k == KD - 1))
                    # psum [128 tok, (8 heads)(64)] -> staging strided (65)
                    for j in range(8):
                        h = hh * 8 + j
                        nc.vector.tensor_scalar_mul(
                            stage[:, blk * VSEG + h * (HD + 1):
                                  blk * VSEG + h * (HD + 1) + HD],
                            pv[:, ts(j, HD)], 1.0)
            ag_in = dram.tile([128, 2 * VSEG], BF16, tag="agiv",
                              name=f"agiv{l}")
            ag_out = dram.tile([G, 128, 2 * VSEG], BF16, tag="agov",
                               name=f"agov{l}")
            nc.sync.dma_start(out=ag_in[:], in_=stage[:])
            nc.gpsimd.collective_compute(
                "AllGather", ALU.bypass, replica_groups=REPLICA_GROUPS,
                ins=[ag_in.opt()], outs=[ag_out.opt()])
            return ag_out

        def q_proj(l, n1):
            q = qp.tile([128, KD, TPC], BF16, tag="q", name=f"q{l}")
            for mp in range(KD // 2):
                pq = ps_mm.tile([128, 2, TPC], F32, tag="mm",
                                name=f"pq{l}_{mp}")
                for j in range(2):
                    m = 2 * mp + j
                    wqt = wq_p.tile([128, KD, 128], BF16, tag="wqk",
                                    name=f"wq{l}_{m}")
                    nc.sync.dma_start(out=wqt[:], in_=inp[_f("wq", l)][:, m])
                    for k in range(KD):
                        nc.tensor.matmul(pq[:, j, :], wqt[:, k, :],
                                         n1[:, k, :],
                                         start=(k == 0), stop=(k == KD - 1))
                    nc.scalar.copy(q[:, m, :], pq[:, j, :])
            return q

        def attention(l, q, kg, vg, oT):
            def phase1(h):
                pp = 64 * (h % 2)
                hc = h // 2
                eTile = ep.tile([128, NSLOT, BLK], BF16, tag="eT",
                                name=f"eT{l}_{h}")
                # scores (transposed [k, q]) + exp + causal mask,
                # batched 4 slots per PSUM bank
                for grp in range(NSLOT // 4):
                    pss = ps_mm.tile([128, 4, BLK], F32, tag="mm",
                                     name=f"sc{l}_{h}_{grp}")
                    for j in range(4):
                        s = 4 * grp + j
                        kb = s if s < 4 else s - 4
                        qc = ts(0, BLK) if s < 4 else ts(1, BLK)
                        rho, sg = RHO[kb], SIG[kb]
                        nc.tensor.matmul(
                            pss[:, j, :],
                            kg[pp:pp + 64, rho,
                               hc * TPC + sg * BLK: hc * TPC + sg * BLK + BLK],
                            q[pp:pp + 64, hc, qc], start=True, stop=True)
                    nc.scalar.activation(eTile[:, ts(grp, 4), :], pss[:],
                                         AF.Exp)
                    nc.vector.tensor_tensor(
                        out=eTile[:, ts(grp, 4), :],
                        in0=eTile[:, ts(grp, 4), :],
                        in1=m01t[:, ts(grp, 4), :], op=ALU.mult)
                return eTile

            def phase2(h, eTile):
                pp = 64 * (h % 2)
                hc = h // 2
                # AV (+ ones-column denominator)
                psA = ps_po.tile([HD + 1, BLK], F32, tag="po",
                                 name=f"pa{l}_{h}")
                psB = ps_po.tile([HD + 1, BLK], F32, tag="po",
                                 name=f"pb{l}_{h}")
                for s in range(NSLOT):
                    kb = s if s < 4 else s - 4
                    rho, sg = RHO[kb], SIG[kb]
                    dst = psA if s < 4 else psB
                    vsl = vg[:, rho,
                             sg * VSEG + h * (HD + 1):
                             sg * VSEG + (h + 1) * (HD + 1)]
                    nc.tensor.matmul(dst[:], vsl, eTile[:, s, :],
                                     start=(s in (0, 4)),
                                     stop=(s in (3, NSLOT - 1)))
                for side, pso, cc in ((0, psA, ts(0, BLK)), (1, psB, ts(1, BLK))):
                    r_ = stat.tile([1, BLK], F32, tag="rr",
                                   name=f"r{l}_{h}_{side}")
                    nc.vector.reciprocal(r_[:], pso[HD:HD + 1, :])
                    bb = bcp.tile([64, BLK], F32, tag="bb",
                                  name=f"bb{l}_{h}_{side}")
                    nc.gpsimd.partition_broadcast(bb[:], r_[:])
                    nc.vector.tensor_tensor(
                        out=oT[pp:pp + 64, hc, cc], in0=pso[0:HD, :],
                        in1=bb[:], op=ALU.mult)

            held = [phase1(h) for h in range(4)]
            for h in range(4):
                phase2(h, held[h])
            for h in range(4, H):
                phase2(h, phase1(h))

        def wo_residual(l, oT, s1, s2):
            for mp in range(KD // 2):
                po = ps_mm.tile([128, 2, TPC], F32, tag="mm",
                                name=f"po{l}_{mp}")
                for j in range(2):
                    m = 2 * mp + j
                    wot = wq_p.tile([128, KD, 128], BF16, tag="wqk",
                                    name=f"wo{l}_{m}")
                    nc.sync.dma_start(out=wot[:], in_=inp[_f("wo", l)][:, m])
                    for k in range(KD):
                        nc.tensor.matmul(po[:, j, :], wot[:, k, :],
                                         oT[:, k, :],
                                         start=(k == 0), stop=(k == KD - 1))
                    nc.vector.tensor_tensor(
                        out=xt[:, m, :], in0=xt[:, m, :].bitcast(F32),
                        in1=po[:, j, :], op=ALU.add)
                for j in range(2):
                    ln_stats_k(xt, s1, s2, 2 * mp + j)

        def ffn(l, n2, s1, s2):
            b1_t = bias.tile([128, M1], F32, tag="bias", name=f"b1{l}")
            nc.sync.dma_start(out=b1_t[:], in_=inp[_f("b1", l)][:])
            b2_t = bias.tile([128, M2], F32, tag="bias", name=f"b2{l}")
            nc.sync.dma_start(out=b2_t[:], in_=inp[_f("b2", l)][:])
            hT = hp.tile([128, M1, TPC], BF16, tag="h", name=f"hT{l}")
            for mp in range(M1 // 2):
                p1 = ps_mm.tile([128, 2, TPC], F32, tag="mm",
                                name=f"p1{l}_{mp}")
                for j in range(2):
                    m = 2 * mp + j
                    w1t = wq_p.tile([128, KD, 128], BF16, tag="wqk",
                                    name=f"w1{l}_{m}")
                    nc.sync.dma_start(out=w1t[:], in_=inp[_f("w1", l)][:, m])
                    for k in range(KD):
                        nc.tensor.matmul(p1[:, j, :], w1t[:, k, :],
                                         n2[:, k, :],
                                         start=(k == 0), stop=(k == KD - 1))
                    nc.scalar.activation(hT[:, m, :], p1[:, j, :], AF.Gelu,
                                         bias=b1_t[:, m:m + 1])
            for mp in range(M2 // 2):
                p2 = ps_mm.tile([128, 2, TPC], F32, tag="mm",
                                name=f"p2{l}_{mp}")
                for j in range(2):
                    m = 2 * mp + j
                    w2t = w2_p.tile([128, M1, 128], BF16, tag="w2",
                                    name=f"w2{l}_{m}")
                    nc.sync.dma_start(out=w2t[:], in_=inp[_f("w2", l)][:, m])
                    for k in range(M1):
                        nc.tensor.matmul(p2[:, j, :], w2t[:, k, :],
                                         hT[:, k, :],
                                         start=(k == 0), stop=(k == M1 - 1))
                    nc.vector.scalar_tensor_tensor(
                        out=xt[:, m, :], in0=p2[:, j, :],
                        scalar=b2_t[:, m:m + 1],
                        in1=xt[:, m, :].bitcast(F32), op0=ALU.add,
                        op1=ALU.add)
                for j in range(2):
                    ln_stats_k(xt, s1, s2, 2 * mp + j)

        def tap_bf16(name, t):
            if name in taps:
                f = stg.tile(list(t.shape), F32, tag="tapf", name="tp" + name)
                nc.scalar.copy(f[:], t[:])
                nc.sync.dma_start(out=taps[name][:], in_=f[:])

        # ---------------- main loop ----------------
        n1 = npool.tile([128, KD, TPC], BF16, tag="n", name="n1_p")
        layer_norm(xt, n1)
        for l in range(L):
            if l == 0:
                tap_bf16("n1_0", n1)
            agk = k_stage_gather(l, n1)
            agv = v_stage_gather(l, n1)
            q = q_proj(l, n1)
            kg = kvp.tile([128, G, KCOLS], BF16, tag="kg", name=f"kg{l}")
            HALF = KCOLS // 2
            for hh in range(2):
                for rho in range(G):
                    nc.sync.dma_start(
                        out=kg[:, rho, ts(hh, HALF)],
                        in_=agk[rho][:, ts(hh, HALF)])
            vg = kvp.tile([128, G, 2 * VSEG], BF16, tag="vg", name=f"vg{l}")
            for sg in range(2):
                for rho in range(G):
                    nc.sync.dma_start(
                        out=vg[:, rho, ts(sg, VSEG)],
                        in_=agv[rho][:, ts(sg, VSEG)])
            if l == 0:
                tap_bf16("q_0", q)
            oT = op.tile([128, KD, TPC], BF16, tag="oT", name=f"oT{l}")
            attention(l, q, kg, vg, oT)
            if l == 0:
                tap_bf16("o_0", oT)
            s1a, s2a = ln_alloc(f"ln2_{l}")
            wo_residual(l, oT, s1a, s2a)
            if l == 0 and "xa_0" in taps:
                nc.sync.dma_start(out=taps["xa_0"][:], in_=xt[:].bitcast(F32))
            n2 = npool.tile([128, KD, TPC], BF16, tag="n", name=f"n2_{l}")
            ln_finish(xt, n2, s1a, s2a)
            s1b, s2b = ln_alloc(f"ln1_{l + 1}")
            ffn(l, n2, s1b, s2b)
            if l == 0 and "x_1" in taps:
                nc.sync.dma_start(out=taps["x_1"][:], in_=xt[:].bitcast(F32))
            n1 = npool.tile([128, KD, TPC], BF16, tag="n", name=f"n1_{l + 1}")
            ln_finish(xt, n1, s1b, s2b)   # LN1 of next layer, or final LN

        nf = n1
        tap_bf16("nf", nf)

        # final AllGather of nf, then vocab-sharded logits
        nf_in = dram.tile([128, NFCOLS], BF16, tag="nfi")
        nf_out = dram.tile([G, 128, NFCOLS], BF16, tag="nfo")
        nc.sync.dma_start(out=nf_in[:], in_=nf[:])
        nc.gpsimd.collective_compute(
            "AllGather", ALU.bypass, replica_groups=REPLICA_GROUPS,
            ins=[nf_in.opt()], outs=[nf_out.opt()])
        nfg = nfp.tile([128, G, KD, TPC], BF16, tag="nfg")
        for rho in range(G):
            nc.sync.dma_start(out=nfg[:, rho], in_=nf_out[rho])

        for vb in range(NVB):
            ebt = embp.tile([128, KD, 128], BF16, tag="emb", name=f"eb{vb}")
            nc.sync.dma_start(out=ebt[:], in_=inp["emb"][:, vb])
            for half in range(2):
                pl = ps_mm.tile([128, 512], F32, tag="mm",
                                name=f"pl{vb}_{half}")
                for k in range(KD):
                    nc.tensor.matmul(pl[:], ebt[:, k, :],
                                     nfg[:, ts(half, 2), k, :],
                                     start=(k == 0), stop=(k == KD - 1))
                lo = lout.tile([128, 512], F32, tag="lo",
                               name=f"lo{vb}_{half}")
                if (vb + half) % 2 == 0:
                    nc.scalar.copy(lo[:], pl[:])
                else:
                    nc.vector.tensor_scalar_mul(lo[:], pl[:], 1.0)
                nc.sync.dma_start(out=logits[ts(vb, 128), ts(half, 512)],
                                  in_=lo[:])


# ------------------------------------------------------------------
# Host side
# ------------------------------------------------------------------

def _kfold(w):
    """[in, out] -> [128, in//128, out]."""
    i, o = w.shape
    return np.ascontiguousarray(w.reshape(i // 128, 128, o).transpose(1, 0, 2))


def _mslice(w):
    """[in, out] -> [128, out//128, in//128, 128] contiguous strips."""
    i, o = w.shape
    t = w.reshape(i // 128, 128, o // 128, 128)
    return np.ascontiguousarray(t.transpose(1, 2, 0, 3))


def _cols(v):
    """[n] -> [128, n//128] per-partition bias columns."""
    return np.ascontiguousarray(v.reshape(-1, 128).T)


def _bf(a):
    return np.ascontiguousarray(a).astype(BF16NP)


def prep_inputs(inputs):
    f = lambda a: np.asarray(a, np.float32)
    tokens = np.asarray(inputs["tokens"])
    tok_emb, pos_emb = f(inputs["tok_emb"]), f(inputs["pos_emb"])
    ln1_g = f(inputs["ln1_g"])
    wq, wk = f(inputs["wq"]), f(inputs["wk"])
    wv, wo = f(inputs["wv"]), f(inputs["wo"])
    ln2_g, ln2_b = f(inputs["ln2_g"]), f(inputs["ln2_b"])
    w1, b1 = f(inputs["w1"]), f(inputs["b1"])
    w2, b2 = f(inputs["w2"]), f(inputs["b2"])
    lnf_g = f(inputs["lnf_g"])

    sc = 1.0 / np.sqrt(HD)
    x0 = tok_emb[tokens] + pos_emb[:S][None]          # [B, S, D]
    ones = np.ones((128, 1), np.float32)

    # shared (identical on all cores) weight tensors
    shared = {"ones": ones}
    for l in range(L):
        shared[_f("wq", l)] = _bf(_mslice(ln1_g[l][:, None] * wq[l] * sc))
        shared[_f("wk", l)] = _bf(_mslice(ln1_g[l][:, None] * wk[l]))
        shared[_f("wv", l)] = _bf(_kfold(ln1_g[l][:, None] * wv[l]))
        shared[_f("wo", l)] = _bf(_mslice(wo[l]))
        shared[_f("w1", l)] = _bf(_mslice(ln2_g[l][:, None] * w1[l]))
        shared[_f("w2", l)] = _bf(_mslice(w2[l]))
        shared[_f("b1", l)] = _cols(b1[l] + ln2_b[l] @ w1[l])
        shared[_f("b2", l)] = _cols(b2[l])

    in_maps = []
    for core in range(N_CORES):
        g, r = core // G, core % G
        A_blk, B_blk = r, 7 - r
        m = dict(shared)
        xo = np.concatenate([x0[g, 128 * A_blk:128 * A_blk + 128],
                             x0[g, 128 * B_blk:128 * B_blk + 128]], 0)
        m["x0"] = _kfold(np.ascontiguousarray(xo.T))
        m01 = np.zeros((128, NSLOT, BLK), np.float32)
        kp = np.arange(128)[:, None]
        qf = np.arange(128)[None, :]
        for s in range(NSLOT):
            qb = A_blk if s < 4 else B_blk
            kb = s if s < 4 else s - 4
            m01[:, s, :] = (128 * kb + kp <= 128 * qb + qf)
        m["m01"] = _bf(m01)
        v0 = r * VS
        v1 = min(v0 + VS, V)
        epad = np.zeros((D, VSP), np.float32)
        epad[:, :v1 - v0] = (tok_emb[v0:v1] * lnf_g[None, :]).T
        m["emb"] = _bf(_mslice(epad))
        in_maps.append(m)
    return in_maps


_CACHED = {}


def _get_program(debug_taps=False):
    key = bool(debug_taps)
    if key not in _CACHED:
        _CACHED[key] = build_program(debug_taps)
    return _CACHED[key]


def run(inputs, debug_taps=False, trace=False, **kw):
    nc = _get_program(debug_taps)
    in_maps = prep_inputs(inputs)
    return run_bass_kernel_spmd(nc, in_maps, list(range(N_CORES)),
                                trace=trace, **kw)


# token column -> natural token index within a group's 1024 tokens
def _colperm():
    perm = np.empty(T, np.int64)
    for c in range(T):
        rho, rem = divmod(c, 256)
        half, qf = divmod(rem, 128)
        blkid = rho if half == 0 else 7 - rho
        perm[c] = 128 * blkid + qf
    return perm


def assemble(results, inputs):
    lnf_b = np.asarray(inputs["lnf_b"], np.float32)
    tok_emb = np.asarray(inputs["tok_emb"], np.float32)
    perm = _colperm()
    out = np.empty((B, S, V), np.float32)
    for b in range(B):
        for r in range(G):
            v0 = r * VS
            v1 = min(v0 + VS, V)
            part = results[b * G + r]["logits"][:v1 - v0, :]  # [rows, T]
            out[b, perm, v0:v1] = part.T
    if np.any(lnf_b):
        out += (tok_emb @ lnf_b)[None, None, :]
    return out


def kernel(**inputs):
    res = run(inputs)
    return assemble(res.results, inputs)


if __name__ == "__main__":
    print("building program...")
    build_program()
    print("build + compile OK")

